# revision 21
# baseline (speedup 1.0000x reference)
"""HSTU-style attention block (RoPE + multi-scale temporal agg + SDPA + LN + out-proj)
for Trainium2, data-parallel over batch across 8 NeuronCores.

v2 design notes (vs the f32r baseline):
  - all matmul operands are bf16 (PSUM accumulation stays f32); halves DMA+SBUF
  - temporal aggregation for V is folded into the host-side input prep
    (T @ Xv before the projection -- T and the projection commute for V)
  - LayerNorm gamma/beta fold into Wo/bo on the host, so device LN is sub+mult
  - Q^T/K^T stay resident in SBUF (no DRAM spill/reload, no zero-pad DMA);
    per-head K tiles carry persistent zero halves so the scores contraction
    stays K=128 (keeps the PE activity monitor/clock happy)
  - temporal agg runs on 128-wide output windows (banded T: 2-3 contraction
    chunks instead of 5)
  - softmax Exp is batched [128, 1024] across two PSUM banks (scalar engine
    throughput is co-critical with the PE in the attention phase)
  - t_agg of chunk hc is interleaved right before attention heads 2hc/2hc+1
    to keep the tensor engine continuously busy (p-state ramps to max after
    ~3us of uninterrupted execution)
"""

import numpy as np
import ml_dtypes
import concourse.mybir as mybir
import concourse.tile as tile
from concourse import bacc
from concourse.bass_utils import run_bass_kernel_spmd

B, S, H, NH = 8, 1024, 1024, 16
HD = H // NH  # 64
P = 128
SO = S // P  # 8
HO = H // P  # 8
N_SCALES = 4
LN_EPS = 1e-5
F32 = mybir.dt.float32
BF16 = mybir.dt.bfloat16
BF = ml_dtypes.bfloat16
AF = mybir.ActivationFunctionType

N_CORES = 8
BAND = 12  # T[s', s] == 0 for |s' - s| > 11 (structural)


# ---------------------------------------------------------------- host helpers
def _softmax_np(x):
    x = np.asarray(x, np.float64)
    e = np.exp(x - x.max())
    return e / e.sum()


def _temporal_matrix(temporal_weights):
    """[S, S] matrix T with (T @ x) == temporal_agg(x) along the sequence axis."""
    w = _softmax_np(temporal_weights)
    T = np.eye(S, dtype=np.float64) * w[0]
    for scale in range(1, N_SCALES):
        p = max(1, S // (2 ** scale))
        k = S // p
        pool = np.zeros((p, S), dtype=np.float64)
        for j in range(p):
            pool[j, j * k:(j + 1) * k] = 1.0 / k
        coord = (np.arange(S, dtype=np.float64) + 0.5) * (p / S) - 0.5
        coord = np.clip(coord, 0.0, None)
        i0 = np.minimum(np.floor(coord).astype(np.int64), p - 1)
        i1 = np.minimum(i0 + 1, p - 1)
        lam = (coord - i0).astype(np.float32).astype(np.float64)
        interp = np.zeros((S, p), dtype=np.float64)
        interp[np.arange(S), i0] += 1.0 - lam
        interp[np.arange(S), i1] += lam
        T += w[scale] * (interp @ pool)
    return T.astype(np.float32)


def _apply_T_banded(T, x):
    """T @ x exploiting the +-BAND band structure of T. x: [S, D] f32."""
    out = np.zeros_like(x)
    idx = np.arange(S)
    for d in range(-BAND, BAND + 1):
        s0, s1 = max(0, -d), min(S, S - d)
        diag = T[idx[s0:s1], idx[s0:s1] + d][:, None]
        out[s0:s1] += diag * x[s0 + d:s1 + d]
    return out


def _rope_tables():
    inv_freq = 1.0 / (10000.0 ** (np.arange(0, HD, 2, dtype=np.float64) / HD))
    freqs = np.arange(S, dtype=np.float64)[:, None] * inv_freq[None, :]
    cos = np.repeat(np.cos(freqs), 2, axis=-1).astype(np.float32)  # [S, HD]
    sin = np.repeat(np.sin(freqs), 2, axis=-1).astype(np.float32)
    return cos, sin


def _nat(x):
    """[S, D] -> [P, S//P, D] with x[so*P+p, d] = out[p, so, d]."""
    return np.ascontiguousarray(x.reshape(SO, P, x.shape[-1]).transpose(1, 0, 2))


def _xt_chunks(x):
    """[S, H] -> [P, SO, HO*P] with out[p, so, ho*P + i] = x[so*P + i, ho*P + p]."""
    return np.ascontiguousarray(
        x.reshape(SO, P, HO, P).transpose(3, 0, 2, 1).reshape(P, SO, H))


# ---------------------------------------------------------------- bass program
def _build_program():
    nc = bacc.Bacc("TRN2", target_bir_lowering=False, debug=False)

    d_xt = {a: nc.dram_tensor(f"xt_{a}", [P, SO, H], BF16, kind="ExternalInput")
            for a in ("v", "q", "k")}
    d_w = {a: nc.dram_tensor(f"w_{a}", [P, HO, H], BF16, kind="ExternalInput")
           for a in ("v", "q", "k", "o")}
    d_b = {a: nc.dram_tensor(f"b_{a}", [1, H], F32, kind="ExternalInput")
           for a in ("v", "q", "k", "o")}
    d_tt = nc.dram_tensor("tt", [P, SO, S], BF16, kind="ExternalInput")
    d_cos = nc.dram_tensor("cos_t", [P, SO, HD], BF16, kind="ExternalInput")
    d_sin = nc.dram_tensor("sin_t", [P, SO, HD], BF16, kind="ExternalInput")
    d_y = nc.dram_tensor("y", [P, SO, H], F32, kind="ExternalOutput")

    with tile.TileContext(nc) as tc:
        with (
            tc.tile_pool(name="const", bufs=1) as cpool,
            tc.tile_pool(name="wch", bufs=8) as wpool,
            tc.tile_pool(name="xtp", bufs=2) as xtpool,
            tc.tile_pool(name="big", bufs=2) as bigpool,
            tc.tile_pool(name="ep", bufs=3) as epool,
            tc.tile_pool(name="rot", bufs=2) as rotpool,
            tc.tile_pool(name="rows", bufs=2) as rowspool,
            tc.tile_pool(name="rb", bufs=1) as rbpool,
            tc.tile_pool(name="tmp", bufs=2) as tmppool,
            tc.tile_pool(name="psA", bufs=3, space="PSUM") as psA,
            tc.tile_pool(name="psB", bufs=2, space="PSUM") as psB,
        ):
            # ---- persistent state
            cos_t = cpool.tile([P, SO, HD], BF16, name="cos_t")
            sin_t = cpool.tile([P, SO, HD], BF16, name="sin_t")
            nc.sync.dma_start(cos_t[:], d_cos.ap())
            nc.sync.dma_start(sin_t[:], d_sin.ap())
            tt_t = cpool.tile([P, SO, S], BF16, name="tt_t")
            bb = cpool.tile([P, H], F32, name="bb")
            onesm = cpool.tile([P, 1], BF16, name="onesm")
            nc.vector.memset(onesm[:], 1.0 / H)
            eps_t = cpool.tile([P, 1], F32, name="eps_t")
            nc.vector.memset(eps_t[:], LN_EPS)
            v_ext = cpool.tile([P, SO, NH, HD + 1], BF16, name="v_ext")
            nc.vector.memset(v_ext[:, :, :, HD:HD + 1], 1.0)
            qT = [cpool.tile([P, S], BF16, name=f"qT{hc}") for hc in range(HO)]
            kh = [cpool.tile([P, S], BF16, name=f"kh{h}") for h in range(NH)]
            for h in range(NH):
                # persistent zero halves: the scores stationary stays K=128
                if h % 2 == 0:
                    nc.vector.memset(kh[h][HD:P, :], 0.0)
                else:
                    nc.vector.memset(kh[h][0:HD, :], 0.0)
            attn_T = cpool.tile([P, HO, S], BF16, name="attn_T")
            sq_T = cpool.tile([P, HO, S], BF16, name="sq_T")
            mu_b = cpool.tile([P, S], F32, name="mu_b")
            rstd_b = cpool.tile([P, S], F32, name="rstd_b")
            # preload the Exp activation table off the critical path
            warm = cpool.tile([1, 8], F32, name="warm")
            nc.vector.memset(warm[:], 0.0)
            nc.scalar.activation(warm[:], warm[:], AF.Exp, scale=0.0)

            def load_bias(a):
                brow = rowspool.tile([1, H], F32, tag="brow", bufs=1,
                                     name=f"brow_{a}")
                nc.sync.dma_start(brow[:], d_b[a].ap())
                nc.gpsimd.partition_broadcast(bb[:], brow[:])

            def _rope_chunk(a_nat, so):
                ch = a_nat[:, so, :]
                ch3 = ch.rearrange("p (nh d) -> p nh d", d=HD)
                ch4 = ch.rearrange("p (nh hf dd) -> p nh hf dd", hf=2, dd=HD // 2)
                rot = rotpool.tile([P, H], BF16, tag="rot", name="rot")
                rot4 = rot[:].rearrange("p (nh hf dd) -> p nh hf dd",
                                        hf=2, dd=HD // 2)
                rot3 = rot[:].rearrange("p (nh d) -> p nh d", d=HD)
                # rotate_half on the scalar engine (idle in phase 1)
                nc.scalar.mul(rot4[:, :, 0, :], ch4[:, :, 1, :], -1.0)
                nc.scalar.copy(rot4[:, :, 1, :], ch4[:, :, 0, :])
                cb = cos_t[:, so, :][:, None, :].to_broadcast((P, NH, HD))
                sb = sin_t[:, so, :][:, None, :].to_broadcast((P, NH, HD))
                nc.vector.tensor_tensor(ch3[:], ch3[:], cb, mybir.AluOpType.mult)
                nc.vector.tensor_tensor(rot3[:], rot3[:], sb, mybir.AluOpType.mult)
                nc.vector.tensor_tensor(ch[:], ch[:], rot[:], mybir.AluOpType.add)

            def project(a, dest, rope=False):
                """dest = X_a @ W_a + b_a (dest: a_nat tile or v_ext)."""
                # first x chunk ahead of the weights in the (single) DMA queue
                xc0 = xtpool.tile([P, HO, P], BF16, tag="xt", name=f"x_{a}0")
                nc.sync.dma_start(xc0[:], d_xt[a].ap()[:, 0, :])
                wt = []
                for ko in range(HO):
                    w = wpool.tile([P, H], BF16, tag="w", name=f"w_{a}{ko}")
                    nc.sync.dma_start(w[:], d_w[a].ap()[:, ko, :])
                    wt.append(w)
                load_bias(a)
                for so in range(SO):
                    if so == 0:
                        xc = xc0
                    else:
                        xc = xtpool.tile([P, HO, P], BF16, tag="xt",
                                         name=f"x_{a}{so}")
                        nc.sync.dma_start(xc[:], d_xt[a].ap()[:, so, :])
                    ps = psA.tile([P, S], F32, tag="A", name=f"pp_{a}{so}")
                    for h2 in range(2):
                        for ko in range(HO):
                            nc.tensor.matmul(
                                ps[:, h2 * 512:(h2 + 1) * 512], xc[:, ko, :],
                                wt[ko][:, h2 * 512:(h2 + 1) * 512],
                                start=(ko == 0), stop=(ko == HO - 1),
                                skip_group_check=True)
                    if dest is None:  # V: write straight into v_ext slots
                        ps3 = ps[:].rearrange("p (nh d) -> p nh d", d=HD)
                        bb3 = bb[:].rearrange("p (nh d) -> p nh d", d=HD)
                        nc.vector.tensor_tensor(
                            v_ext[:, so, :, 0:HD], ps3, bb3, mybir.AluOpType.add)
                    else:
                        nc.vector.tensor_tensor(
                            dest[:, so, :], ps[:], bb[:], mybir.AluOpType.add)
                        if rope:
                            _rope_chunk(dest, so)

            def t_agg_chunk(src, hc, kmode):
                """(T @ src).T for dim-chunk hc -> qT[hc] or kh[2hc]/kh[2hc+1].

                128-wide output windows; banded T needs only chunks
                {w-1, w, w+1} of the contraction. Four windows share one
                PSUM bank (independent accumulation groups per col range).
                Tiles come from psA (not psB) so they never wait on the
                previous head's PV eviction chain."""
                for g in range(2):
                    ps = psA.tile([P, S], F32, tag="A", name=f"tg_{hc}{g}")
                    for wi in range(4):
                        w = g * 4 + wi
                        sos = [x for x in (w - 1, w, w + 1) if 0 <= x < SO]
                        for i, so in enumerate(sos):
                            nc.tensor.matmul(
                                ps[:, wi * P:(wi + 1) * P],
                                src[:, so, hc * P:(hc + 1) * P],
                                tt_t[:, so, w * P:(w + 1) * P],
                                start=(i == 0), stop=(i == len(sos) - 1),
                                skip_group_check=True)
                    cols = slice(g * 512, (g + 1) * 512)
                    # two-chunk-ahead pipelining gives these DVE evictions a
                    # full head of slack; keeping them off the scalar engine
                    # preserves its exp throughput (the hc-loop limiter)
                    if kmode:
                        nc.vector.tensor_copy(kh[2 * hc][0:HD, cols],
                                              ps[0:HD, 0:512])
                        nc.vector.tensor_copy(kh[2 * hc + 1][HD:P, cols],
                                              ps[HD:P, 0:512])
                    else:
                        nc.vector.tensor_copy(qT[hc][:, cols], ps[:, 0:512])

            def attn_head(h):
                hc, off = h // 2, (h % 2) * HD
                khh = kh[h]
                pv = [psB.tile([P, 512], F32, tag="B", name=f"pv{h}_{q2}")
                      for q2 in range(2)]
                es = []

                def pv_step(j):
                    for q2 in range(2):
                        nc.tensor.matmul(
                            pv[q2][0:HD + 1, :], v_ext[:, j, h, :],
                            es[j][:, q2 * 512:(q2 + 1) * 512],
                            start=(j == 0), stop=(j == SO - 1),
                            skip_group_check=True)

                for kc in range(SO):
                    sp = psA.tile([P, S], F32, tag="A", name=f"sp{h}_{kc}")
                    for q2 in range(2):
                        nc.tensor.matmul(
                            sp[:, q2 * 512:(q2 + 1) * 512],
                            khh[:, kc * P:(kc + 1) * P],
                            qT[hc][:, q2 * 512:(q2 + 1) * 512],
                            start=True, stop=True, skip_group_check=True)
                    e = epool.tile([P, S], BF16, tag="e", name=f"e{h}_{kc}")
                    nc.scalar.activation(e[:], sp[:], AF.Exp, scale=0.125)
                    es.append(e)
                    # PV trails scores by 2 so the PE never spins on the
                    # first Exp of a head
                    if kc > 1:
                        pv_step(kc - 2)
                pv_step(SO - 2)
                pv_step(SO - 1)

                for q2 in range(2):
                    qs = slice(q2 * 512, (q2 + 1) * 512)
                    # one copy evicts the whole PV block -> PSUM slot frees
                    # immediately; the normalization chain works from SBUF
                    # (also: custom-DVE reciprocal misreads PSUM)
                    praw = tmppool.tile([P, 512], F32, tag="t1",
                                        name=f"praw{h}_{q2}")
                    nc.vector.tensor_copy(praw[0:HD, :], pv[q2][0:HD, :])
                    sraw = rowspool.tile([1, 512], F32, tag="rows",
                                         name=f"sraw{h}_{q2}")
                    nc.vector.tensor_copy(sraw[:], pv[q2][HD:HD + 1, :])
                    srow = rowspool.tile([1, 512], F32, tag="rows",
                                         name=f"srow{h}_{q2}")
                    # custom-DVE reciprocal needs an SBUF input at partition 0
                    nc.vector.reciprocal_approx_fast(srow[:], sraw[:])
                    rb = rbpool.tile([HD, 512], F32, tag="rb", name=f"rb{h}_{q2}")
                    nc.gpsimd.partition_broadcast(rb[:], srow[:])
                    dst = attn_T[off:off + HD, hc, qs]
                    nc.vector.tensor_tensor(dst, praw[0:HD, :], rb[:],
                                            mybir.AluOpType.mult)
                    nc.vector.tensor_tensor(sq_T[off:off + HD, hc, qs], dst, dst,
                                            mybir.AluOpType.mult)

            # ---- phase 1: projections (+RoPE for Q/K)
            project("v", None)
            a_q = bigpool.tile([P, SO, H], BF16, tag="big", name="a_q")
            project("q", a_q, rope=True)
            a_k = bigpool.tile([P, SO, H], BF16, tag="big", name="a_k")
            project("k", a_k, rope=True)
            # tt rides the idle DMA window during proj-K compute
            nc.sync.dma_start(tt_t[:], d_tt.ap())

            # ---- phase 2: interleaved temporal agg + attention
            # t_agg runs two chunks ahead of the heads that consume it, so
            # its evictions never gate the next head's scores matmuls
            load_bias("o")  # bb free until the output projection
            t_agg_chunk(a_q, 0, kmode=False)
            t_agg_chunk(a_k, 0, kmode=True)
            t_agg_chunk(a_q, 1, kmode=False)
            t_agg_chunk(a_k, 1, kmode=True)
            wo_t = None
            for hc in range(HO):
                # t_agg filler before each head covers its exp spin-up
                if hc + 2 < HO:
                    t_agg_chunk(a_q, hc + 2, kmode=False)
                elif hc + 2 == HO:
                    # a_q fully consumed: its big-pool slot takes Wo
                    wo_t = bigpool.tile([P, HO, H], BF16, tag="big", name="wo_t")
                    nc.sync.dma_start(wo_t[:], d_w["o"].ap())
                attn_head(2 * hc)
                if hc + 2 < HO:
                    t_agg_chunk(a_k, hc + 2, kmode=True)
                attn_head(2 * hc + 1)

            # ---- phase 3: LN stats + apply + output projection, by s-halves
            def ln_stats(half):
                # one psA tile: mu group in cols 0:512, ms group in 512:1024
                # (psB would wait on the last head's PV eviction chain)
                cols = slice(half * 512, (half + 1) * 512)
                st = psA.tile([P, S], F32, tag="A", name=f"st{half}")
                for hc in range(HO):
                    nc.tensor.matmul(st[0:1, 0:512], onesm[:],
                                     attn_T[:, hc, cols],
                                     start=(hc == 0), stop=(hc == HO - 1),
                                     skip_group_check=True)
                for hc in range(HO):
                    nc.tensor.matmul(st[0:1, 512:1024], onesm[:],
                                     sq_T[:, hc, cols],
                                     start=(hc == 0), stop=(hc == HO - 1),
                                     skip_group_check=True)
                return st[0:1, 0:512], st[0:1, 512:1024]

            def ln_chain(half, mu_ps, ms_ps):
                cols = slice(half * 512, (half + 1) * 512)
                mu_row = rowspool.tile([1, 512], F32, tag="rows",
                                       name=f"mur{half}")
                nc.vector.tensor_copy(mu_row[:], mu_ps[0:1, :])
                nc.gpsimd.partition_broadcast(mu_b[:, cols], mu_row[:])
                m2 = rowspool.tile([1, 512], F32, tag="rows", name=f"m2r{half}")
                nc.scalar.square(m2[:], mu_row[:])
                nc.vector.tensor_tensor(m2[:], ms_ps[0:1, :], m2[:],
                                        mybir.AluOpType.subtract)
                nc.scalar.activation(m2[:], m2[:], AF.Sqrt, bias=eps_t[0:1, :])
                rs = rowspool.tile([1, 512], F32, tag="rows", name=f"rsr{half}")
                nc.vector.reciprocal_approx_fast(rs[:], m2[:])
                nc.gpsimd.partition_broadcast(rstd_b[:, cols], rs[:])
                for hc in range(HO):
                    t1 = tmppool.tile([P, 512], F32, tag="t1", name=f"t1_{half}{hc}")
                    nc.vector.tensor_tensor(t1[:], attn_T[:, hc, cols],
                                            mu_b[:, cols],
                                            mybir.AluOpType.subtract)
                    nc.vector.tensor_tensor(attn_T[:, hc, cols], t1[:],
                                            rstd_b[:, cols],
                                            mybir.AluOpType.mult)

            def out_proj(half):
                for so in range(half * 4, half * 4 + 4):
                    ps = psA.tile([P, S], F32, tag="A", name=f"op{so}")
                    for h2 in range(2):
                        for hc in range(HO):
                            nc.tensor.matmul(
                                ps[:, h2 * 512:(h2 + 1) * 512],
                                attn_T[:, hc, so * P:(so + 1) * P],
                                wo_t[:, hc, h2 * 512:(h2 + 1) * 512],
                                start=(hc == 0), stop=(hc == HO - 1),
                                skip_group_check=True)
                    for h2 in range(2):
                        hs = slice(h2 * 512, (h2 + 1) * 512)
                        yev = tmppool.tile([P, 512], F32, tag="y",
                                           name=f"y{so}_{h2}")
                        nc.vector.tensor_tensor(yev[:], ps[:, hs], bb[:, hs],
                                                mybir.AluOpType.add)
                        nc.sync.dma_start(d_y.ap()[:, so, hs], yev[:])

            mu0, ms0 = ln_stats(0)
            ln_chain(0, mu0, ms0)
            mu1, ms1 = ln_stats(1)  # PE work while DVE applies half 0
            out_proj(0)
            ln_chain(1, mu1, ms1)
            out_proj(1)

    nc.compile()
    return nc


_NC = None


def _get_nc():
    global _NC
    if _NC is None:
        _NC = _build_program()
    return _NC


def _host_inputs(query, key, value, Wq, bq, Wk, bk, Wv, bv, Wo, bo,
                 temporal_weights, ln_gamma, ln_beta):
    T = _temporal_matrix(temporal_weights)
    tt_host = np.ascontiguousarray(  # TT[p, so, s'] = T[s', so*P+p]
        T.T.reshape(SO, P, S).transpose(1, 0, 2)).astype(BF)
    cos, sin = _rope_tables()
    gamma = np.asarray(ln_gamma, np.float64)
    beta = np.asarray(ln_beta, np.float64)
    Wo64 = np.asarray(Wo, np.float64)
    Wo_f = (Wo64 * gamma[:, None]).astype(np.float32)   # fold LN gamma
    bo_f = (beta @ Wo64 + np.asarray(bo, np.float64)).astype(np.float32)
    common = {
        "w_v": _nat(np.asarray(Wv, np.float32)).astype(BF),
        "w_q": _nat(np.asarray(Wq, np.float32)).astype(BF),
        "w_k": _nat(np.asarray(Wk, np.float32)).astype(BF),
        "w_o": _nat(Wo_f).astype(BF),
        "b_v": np.asarray(bv, np.float32).reshape(1, H),
        "b_q": np.asarray(bq, np.float32).reshape(1, H),
        "b_k": np.asarray(bk, np.float32).reshape(1, H),
        "b_o": bo_f.reshape(1, H),
        "tt": tt_host,
        "cos_t": _nat(cos).astype(BF),
        "sin_t": _nat(sin).astype(BF),
    }
    in_maps = []
    for c in range(N_CORES):
        m = dict(common)
        m["xt_q"] = _xt_chunks(np.asarray(query[c], np.float32)).astype(BF)
        m["xt_k"] = _xt_chunks(np.asarray(key[c], np.float32)).astype(BF)
        m["xt_v"] = _xt_chunks(
            _apply_T_banded(T, np.asarray(value[c], np.float32))).astype(BF)
        in_maps.append(m)
    return in_maps


def kernel(query, key, value, Wq, bq, Wk, bk, Wv, bv, Wo, bo,
           temporal_weights, ln_gamma, ln_beta):
    in_maps = _host_inputs(query, key, value, Wq, bq, Wk, bk, Wv, bv, Wo, bo,
                           temporal_weights, ln_gamma, ln_beta)
    nc = _get_nc()
    res = run_bass_kernel_spmd(nc, in_maps, list(range(N_CORES)))
    out = np.empty((B, S, H), np.float32)
    for c in range(N_CORES):
        y = res.results[c]["y"]  # [P, SO, H]
        out[c] = y.transpose(1, 0, 2).reshape(S, H)
    return out
